# revision 40
# baseline (speedup 1.0000x reference)
"""GAT (graph attention) Trainium2 kernel.

Full-input contract: kernel(**inputs) takes the unsharded tensors
  x   (8, 1024, 512) f32
  adj (8, 1024, 1024) i32
  W   (8, 256, 512) f32
  a1  (8, 256) f32
  a2  (8, 256) f32
and returns out (8, 1024, 256) f32.

Sharding: data-parallel over batch B=8 across the 8 NeuronCores; each core
computes all heads for one batch element. No collectives needed.

Per-core algorithm (N=1024 nodes, F_in=512, F_out=256, H=8 heads), all in
the transposed attention layout e^T[j,i] = f1[i] + f2[j] so that att^T is
directly the matmul lhsT (adjacency transposed once instead of per-head
attention transposes):
  h_h   = x @ W_h^T              (fp8e4m3 DoubleRow PE matmul, W scaled by
                                  16 into fp8 range, unscaled at PSUM evac)
  f1/f2 = x @ (W_h^T a)          (fp32 PE matmul, exact)
  exp(lrelu(v)) = max(exp(v), exp(0.2 v)):
      exp(v)     on ACT (exact; dominates softmax where it matters)
      exp(0.2 v) via bf16 Schraudolph bit-trick on GPSIMD (~3% rel err on
                 weights <= 1 only; end-to-end ~4e-4)
      max + adjacency mask on DVE in bf16 packed 2x mode, jt-pair spans
  o = att @ [h | 1]              (PE; ones column gives softmax denominator
                                  for free, no max-subtraction needed)
  elu(o/d) + 1 = min(exp(o/d),1) + max(o/d,0)  (+1 cancels: log_softmax is
                                  shift invariant; exp on ACT, relu+min/add
                                  on DVE)
  out = log_softmax(sum_h elu_h) (ACT Exp with accum_out, batched Ln)
"""
import sys

sys.path.insert(0, "/opt/trn_rl_repo")

from contextlib import ExitStack

import numpy as np

import concourse.bacc as bacc
import concourse.bass as bass
import concourse.mybir as mybir
import concourse.tile as tile
from concourse import masks
from concourse._compat import with_exitstack

F32 = mybir.dt.float32
BF16 = mybir.dt.bfloat16
FP8 = mybir.dt.float8e4
I32 = mybir.dt.int32
I16 = mybir.dt.int16
AF = mybir.ActivationFunctionType
ALU = mybir.AluOpType
DR = mybir.MatmulPerfMode.DoubleRow

N, F_IN, F_OUT, H, B = 1024, 512, 256, 8, 8
P = 128
NT = N // P        # 8 node tiles
FT = F_IN // P     # 4 f_in tiles
OT = F_OUT // P    # 2 f_out tiles
HB = F_OUT + 2     # per-head block in h_ext: 256 values + ones col + pad
WS = 16.0          # fp8 scale for W


@with_exitstack
def gat_kernel(ctx: ExitStack, tc, out_d, x_d, adj_d, W_d, a1_d, a2_d,
               variant=()):
    nc = tc.nc
    variant = set(variant)
    # dev-only stage gating for HW ablation: variant may name the last
    # stage to run ("dma", "A", "A2", "B", "C"); default runs everything.
    stages = ("dma", "A", "A2", "B", "C", "D")
    last = "D"
    for s in stages:
        if s in variant:
            last = s
    upto = stages.index(last)

    def on(stage):
        return stages.index(stage) <= upto

    def dummy_out():
        zz = ctx.enter_context(tc.tile_pool(name="dummy", bufs=1)).tile(
            [P, F_OUT], F32, name="zo", tag="zo")
        nc.vector.memset(zz[:], 0.0)
        for it in range(NT):
            nc.sync.dma_start(out_d[it * P:(it + 1) * P, :], zz[:])

    if upto == 0:
        # DMA-only floor: load every input, write a dummy output. The
        # "live" flavor folds one column of every loaded tile into the
        # output so the compiler cannot dead-code the loads.
        live = "live" in variant
        dp0 = ctx.enter_context(tc.tile_pool(name="dmaonly", bufs=4))
        lp = ctx.enter_context(tc.tile_pool(name="dmalive", bufs=1))
        acc = lp.tile([P, 40], F32, name="acc", tag="acc")
        k = 0
        for nt in range(NT):
            t = dp0.tile([P, F_IN], F32, name="xl", tag="xl")
            nc.sync.dma_start(t[:], x_d[nt * P:(nt + 1) * P, :])
            if live:
                nc.vector.tensor_copy(acc[:, k:k + 1], t[:, 0:1]); k += 1
        for it in range(NT):
            t = dp0.tile([P, N], I32, name="al", tag="al")
            nc.sync.dma_start(t[:], adj_d[it * P:(it + 1) * P, :])
            if live:
                nc.vector.tensor_scalar(acc[:, k:k + 1], t[:, 0:1], 0, None,
                                        op0=ALU.add); k += 1
        for h in range(H):
            for ot in range(OT):
                t = dp0.tile([P, F_IN], F32, name="wl", tag="wl")
                nc.sync.dma_start(t[:], W_d[h, ot * P:(ot + 1) * P, :])
                if live:
                    nc.vector.tensor_copy(acc[:, k:k + 1], t[:, 0:1]); k += 1
        if live:
            zz = lp.tile([P, F_OUT], F32, name="zl", tag="zl")
            nc.vector.memset(zz[:], 0.0)
            nc.vector.tensor_copy(zz[:, 0:40], acc[:])
            for it in range(NT):
                nc.sync.dma_start(out_d[it * P:(it + 1) * P, :], zz[:])
        else:
            dummy_out()
        return

    const = ctx.enter_context(tc.tile_pool(name="const", bufs=1))
    ident = const.tile([P, P], F32, name="ident", tag="ident")
    masks.make_identity(nc, ident[:])
    ident_bf = const.tile([P, P], BF16, name="ident_bf", tag="ident_bf")
    masks.make_identity(nc, ident_bf[:])

    persist = ctx.enter_context(tc.tile_pool(name="persist", bufs=1))
    # fp8 DoubleRow operand layout: [128, 2, *] where dim1 indexes the two
    # contraction sub-tiles (fc pair 2q, 2q+1).
    xT8 = [persist.tile([P, 2, N], FP8, name=f"xT8{q}", tag=f"xT8{q}")
           for q in range(FT // 2)]
    WT8 = [persist.tile([P, 2, H * F_OUT], FP8, name=f"WT8{q}", tag=f"WT8{q}")
           for q in range(FT // 2)]
    h_ext = [persist.tile([P, H * HB], BF16, name=f"hext{nt}", tag=f"hext{nt}")
             for nt in range(NT)]
    # adjacency^T in jt-pair spans [128, 2, N] (s = jt % 2)
    adjT = [persist.tile([P, 2, N], BF16, name=f"adjT{q}", tag=f"adjT{q}")
            for q in range(NT // 2)]
    f12 = [persist.tile([P, 16], F32, name=f"f12_{nt}", tag=f"f12_{nt}")
           for nt in range(NT)]
    # f1 rows per head as partition-0 rows (broadcast sources), bf16:
    # raw f1, exp(f1), exp(0.2 f1)
    f1flat = persist.tile([1, H * N], BF16, name="f1flat", tag="f1flat")
    ef1flat = persist.tile([1, H * N], BF16, name="ef1flat", tag="ef1flat")
    ef1flat02 = persist.tile([1, H * N], BF16, name="ef1flat02",
                             tag="ef1flat02")
    ef2 = [persist.tile([P, 16], F32, name=f"ef2_{jt}", tag=f"ef2_{jt}")
           for jt in range(NT)]  # cols 0..7 exp(f2), 8..15 exp(0.2 f2)
    s_acc = [persist.tile([P, F_OUT], F32, name=f"sacc{it}", tag=f"sacc{it}")
             for it in range(NT)]

    # ---------------- Stage A: loads, transposes, f1/f2 ----------------
    with ExitStack() as sa:
        pa = sa.enter_context(tc.tile_pool(name="stageA", bufs=8))
        pa2 = sa.enter_context(tc.tile_pool(name="stageA2", bufs=16))
        xtf_pool = sa.enter_context(tc.tile_pool(name="xtf", bufs=1))
        ps_a = sa.enter_context(tc.tile_pool(name="psA", bufs=3, space="PSUM"))
        ps_aa = sa.enter_context(tc.tile_pool(name="psAa", bufs=1, space="PSUM"))
        ps_aw = sa.enter_context(tc.tile_pool(name="psAw", bufs=1, space="PSUM"))
        ps_af = sa.enter_context(tc.tile_pool(name="psAf", bufs=2, space="PSUM"))

        xT_f32 = [xtf_pool.tile([P, N], F32, name=f"xTf32{fc}", tag=f"xTf32{fc}")
                  for fc in range(FT)]
        w12_sb = xtf_pool.tile([P, 64], F32, name="w12", tag="w12")
        a12_sb = xtf_pool.tile([16, F_OUT], F32, name="a12", tag="a12")

        # a1/a2 -> (16, 256) rows 0..7 = a1 heads, 8..15 = a2 heads
        nc.sync.dma_start(a12_sb[0:8, :], a1_d[:, :])
        nc.sync.dma_start(a12_sb[8:16, :], a2_d[:, :])

        # a12 transpose: (16, 256) -> per ot (128, 16) on partitions
        a12T = xtf_pool.tile([P, 32], F32, name="a12T", tag="a12T")
        for ot in range(OT):
            pt = ps_aa.tile([P, 16], F32, name="psA_a", tag="psA_a")
            nc.tensor.matmul(pt[:], a12_sb[:, ot * P:(ot + 1) * P],
                             ident[0:16, 0:16], is_transpose=True)
            nc.vector.tensor_copy(a12T[:, ot * 16:(ot + 1) * 16], pt[:])
        a12Tv = a12T[:].rearrange("p (t c h) -> p t c h", t=2, c=2)

        # W: load natural, transpose later to WT8; w12 = W^T @ [a1 a2] (fp32)
        w12v = w12_sb[:].rearrange("p (fc c h) -> p fc c h", fc=FT, c=2)
        wnat_all = {}
        for h in range(H):
            wp = ps_aw.tile([P, 8], F32, name="psA_w", tag="psA_w")
            wnats = []
            for ot in range(OT):
                wnat = pa2.tile([P, F_IN], F32, name="wnat", tag="wnat")
                wnats.append(wnat)
                nc.sync.dma_start(wnat[:], W_d[h, ot * P:(ot + 1) * P, :])
            wnat_all[h] = wnats
            for fc in range(FT):
                for ot in range(OT):
                    nc.tensor.matmul(
                        wp[:, fc * 2:(fc + 1) * 2],
                        wnats[ot][:, fc * P:(fc + 1) * P],
                        a12Tv[:, ot, :, h],
                        start=(ot == 0), stop=(ot == OT - 1))
            nc.vector.tensor_copy(w12v[:, :, :, h],
                                  wp[:].rearrange("p (fc c) -> p fc c", fc=FT))

        # x transpose: x (n,f) -> xT (f,n), fp32 + fp8 copies.
        # 4 transposes share one PSUM bank -> 1 wide evacuation each.
        xnats = []
        for nt in range(NT):
            xnat = pa.tile([P, F_IN], F32, name="xnat", tag="xnat")
            nc.sync.dma_start(xnat[:], x_d[nt * P:(nt + 1) * P, :])
            xnats.append(xnat)
        for ntq in range(0, NT, 4):
            for fc in range(FT):
                pt = ps_a.tile([P, 4 * P], F32, name="psA", tag="psA")
                for d in range(4):
                    nc.tensor.matmul(pt[:, d * P:(d + 1) * P],
                                     xnats[ntq + d][:, fc * P:(fc + 1) * P],
                                     ident[:], is_transpose=True)
                nc.scalar.copy(xT_f32[fc][:, ntq * P:(ntq + 4) * P], pt[:])
        for fc in range(FT):
            nc.scalar.copy(xT8[fc // 2][:, fc % 2, :], xT_f32[fc][:])

        # f1/f2 = x @ w12 (fp32): f12[nt] cols = c*8 + h
        for nt in range(NT):
            fp = ps_af.tile([P, 16], F32, name="psA_f", tag="psA_f")
            for fc in range(FT):
                nc.tensor.matmul(fp[:], xT_f32[fc][:, nt * P:(nt + 1) * P],
                                 w12v[:, fc], start=(fc == 0),
                                 stop=(fc == FT - 1))
            nc.vector.tensor_copy(f12[nt][:], fp[:])
            # per-partition exp(f2), exp(0.2 f2) for the separable z paths
            nc.scalar.activation(ef2[nt][:, 0:8], fp[:, 8:16], AF.Exp)
            nc.scalar.activation(ef2[nt][:, 8:16], fp[:, 8:16], AF.Exp,
                                 scale=0.2)
        # f12^T via matmul (w12 as lhsT): [16, N] rows 0..7 = f1 per head;
        # feeds the flat broadcast-source rows in one DMA each.
        f12T = xtf_pool.tile([16, N], BF16, name="f12T", tag="f12T")
        for half in range(2):
            ft = ps_aa.tile([16, N // 2], F32, name="psA_ft", tag="psA_ft")
            for fc in range(FT):
                nc.tensor.matmul(
                    ft[:], w12v[:, fc],
                    xT_f32[fc][:, half * (N // 2):(half + 1) * (N // 2)],
                    start=(fc == 0), stop=(fc == FT - 1))
            nc.vector.tensor_copy(f12T[:, half * (N // 2):(half + 1) * (N // 2)],
                                  ft[:])
        ef1T = xtf_pool.tile([8, N], BF16, name="ef1T", tag="ef1T")
        ef1T02 = xtf_pool.tile([8, N], BF16, name="ef1T02", tag="ef1T02")
        nc.scalar.activation(ef1T[:], f12T[0:8, :], AF.Exp)
        nc.scalar.activation(ef1T02[:], f12T[0:8, :], AF.Exp, scale=0.2)
        nc.sync.dma_start(
            f1flat[0:1, :].rearrange("a (h n) -> a h n", h=H), f12T[0:8, :])
        nc.sync.dma_start(
            ef1flat[0:1, :].rearrange("a (h n) -> a h n", h=H), ef1T[:])
        nc.sync.dma_start(
            ef1flat02[0:1, :].rearrange("a (h n) -> a h n", h=H), ef1T02[:])

        # WT transposes last (needed only by stage B); 16x-scaled fp8 out,
        # 4 blocks per PSUM bank, evacs on the startup-idle ACT engine
        for hp in range(0, H, 2):
            for fc in range(FT):
                pt = ps_a.tile([P, 4 * P], F32, name="psA", tag="psA")
                for dh in range(2):
                    for ot in range(OT):
                        nc.tensor.matmul(
                            pt[:, (dh * 2 + ot) * P:(dh * 2 + ot + 1) * P],
                            wnat_all[hp + dh][ot][:, fc * P:(fc + 1) * P],
                            ident[:], is_transpose=True)
                nc.scalar.activation(
                    WT8[fc // 2][:, fc % 2, hp * F_OUT:(hp + 2) * F_OUT],
                    pt[:], AF.Copy, scale=WS)

    if not on("A2"):
        dummy_out()
        return

    # ---------------- Stage A2: adjacency cast + transpose ----------------
    # 4 it-transposes share a PSUM bank; 1 wide evacuation into jt-pair
    # span tiles.
    with ExitStack() as sb:
        pj = sb.enter_context(tc.tile_pool(name="adjload", bufs=4))
        pjb = sb.enter_context(tc.tile_pool(name="adjcast", bufs=4))
        ps_t = sb.enter_context(tc.tile_pool(name="psT", bufs=4, space="PSUM"))
        for itq in range(0, NT, 4):
            abs_ = []
            for d in range(4):
                it = itq + d
                ai = pj.tile([P, N], I32, name="adji", tag="adji")
                nc.sync.dma_start(ai[:], adj_d[it * P:(it + 1) * P, :])
                ab = pjb.tile([P, N], BF16, name="adjb", tag="adjb")
                if "castdve" in variant:
                    nc.vector.tensor_scalar(ab[:], ai[:], 0, None, op0=ALU.add)
                elif "castpool" in variant:
                    nc.gpsimd.tensor_scalar(ab[:], ai[:], 0, None, op0=ALU.add)
                else:
                    nc.scalar.activation(ab[:], ai[:], AF.Copy)
                abs_.append(ab)
            for jt in range(NT):
                pt = ps_t.tile([P, 4 * P], BF16, name="psT", tag="psT")
                for d in range(4):
                    nc.tensor.matmul(pt[:, d * P:(d + 1) * P],
                                     abs_[d][:, jt * P:(jt + 1) * P],
                                     ident_bf[:], is_transpose=True)
                if "evp" in variant:
                    nc.gpsimd.tensor_copy(
                        adjT[jt // 2][:, jt % 2, itq * P:(itq + 4) * P], pt[:])
                else:
                    nc.scalar.copy(
                        adjT[jt // 2][:, jt % 2, itq * P:(itq + 4) * P], pt[:])

    if not on("B"):
        dummy_out()
        return

    # ---------------- Stage B: h = x @ W^T (fp8 DR), build h_ext ----------
    sb_b = ExitStack()
    ps_h = sb_b.enter_context(tc.tile_pool(name="psH", bufs=4, space="PSUM"))
    for nt in range(NT):
        hv = h_ext[nt][:].rearrange("p (h c) -> p h c", h=H)
        nc.vector.memset(hv[:, :, F_OUT:F_OUT + 1], 1.0)
        for hp in range(H // 2):  # head pairs -> 64 DoubleRow matmuls
            hps = ps_h.tile([P, 2 * F_OUT], F32, name="hpsum", tag="hpsum")
            for q in range(FT // 2):
                nc.tensor.matmul(
                    hps[:], xT8[q][:, :, nt * P:(nt + 1) * P],
                    WT8[q][:, :, hp * 2 * F_OUT:(hp + 1) * 2 * F_OUT],
                    start=(q == 0), stop=(q == FT // 2 - 1), perf_mode=DR)
            # single wide evac, unscaling W's fp8 range factor; alternate
            # engines so neither serializes the psum recycle
            dst = h_ext[nt][:].rearrange("p (h c) -> p h c", h=H)[
                :, 2 * hp:2 * hp + 2, 0:F_OUT]
            if "evp" in variant:
                nc.gpsimd.tensor_scalar(dst, hps[:], 1.0 / WS, None,
                                        op0=ALU.mult)
            elif hp % 2 == 0:
                nc.scalar.activation(dst, hps[:], AF.Copy, scale=1.0 / WS)
            else:
                nc.vector.tensor_scalar(dst, hps[:], 1.0 / WS, None,
                                        op0=ALU.mult)
    sb_b.close()

    if not on("C"):
        dummy_out()
        return

    # ---------------- Stage C: per-head attention ----------------
    # z1 = exp(v) on ACT (exact: dominates softmax for v >= 0).
    # z2 = exp(0.2 v) via a bf16 Schraudolph bit-trick on GPSIMD (~3% rel
    # err; only contributes small weights <= 1, end-to-end impact ~4e-4).
    # max + mask run on DVE over jt-pair spans (2048 elems, bf16 2x mode).
    ps_o = ctx.enter_context(tc.tile_pool(name="psO", bufs=6, space="PSUM"))
    zp = ctx.enter_context(tc.tile_pool(name="zp", bufs=2))
    tmp_p = ctx.enter_context(tc.tile_pool(name="tmp", bufs=3))
    att_p = ctx.enter_context(tc.tile_pool(name="attp", bufs=8))
    ep = ctx.enter_context(tc.tile_pool(name="epilogue", bufs=6))
    f1bp = ctx.enter_context(tc.tile_pool(name="f1bp", bufs=2))

    # Schraudolph constants for bf16: bits = round(A*s*v + B) as int16
    # bitcast to bf16 ~= exp(s*v). B folded per-partition with the f2 bias.
    A1 = (2.0 ** 7) / float(np.log(2.0))
    A02 = A1 * 0.2
    BCONST = 127.0 * 2 ** 7 - 0.043 * 2 ** 7 + 0.49
    z_mode = "zdve" if "zdve" in variant else (
        "zprelu" if "zprelu" in variant else (
            "z2pool" if "z2pool" in variant else "z2act"))
    # separable-z head counts: heads < z1sep_n compute z1 = exp(f1)*exp(f2)
    # as a DVE tensor_scalar instead of an ACT exp (same for z2). Defaults
    # (z1 separable everywhere, epilogue relu on ACT) won the HW sweep.
    z1sep_n, z2sep_n = 8, 0
    rt_act = "rtdve" not in variant
    for v in variant:
        if v.startswith("z1s"):
            z1sep_n = int(v[3:])
        if v.startswith("z2s"):
            z2sep_n = int(v[3:])
    bbp = ctx.enter_context(tc.tile_pool(name="bbp", bufs=1))
    bb = [bbp.tile([P, 8], F32, name=f"bb{jt}", tag=f"bb{jt}")
          for jt in range(NT)]
    for jt in range(NT):
        if z_mode == "z2pool":
            nc.vector.tensor_scalar(bb[jt][:], f12[jt][:, 8:16], A02, BCONST,
                                    op0=ALU.mult, op1=ALU.add)
        else:
            # exact z2 path: bias tiles 0.2*f2 for ACT Exp(0.2 v)
            nc.vector.tensor_scalar(bb[jt][:], f12[jt][:, 8:16], 0.2, None,
                                    op0=ALU.mult)

    def output_stage(h, atts):
        for it in range(NT):
            op = ps_o.tile([P, F_OUT + 1], F32, name="opsum", tag="opsum")
            for jt in range(NT):
                nc.tensor.matmul(op[:],
                                 atts[jt // 2][:, jt % 2, it * P:(it + 1) * P],
                                 h_ext[jt][:, h * HB:h * HB + F_OUT + 1],
                                 start=(jt == 0), stop=(jt == NT - 1))
            rec = ep.tile([P, 1], F32, name="rec", tag="rec")
            nc.vector.reciprocal(rec[:], op[:, F_OUT:F_OUT + 1])
            zt = ep.tile([P, F_OUT], BF16, name="zt", tag="zt")
            nc.scalar.activation(zt[:], op[:, 0:F_OUT], AF.Exp,
                                 scale=rec[:, 0:1])
            rt = ep.tile([P, F_OUT], BF16, name="rt", tag="rt")
            if rt_act:
                nc.scalar.activation(rt[:], op[:, 0:F_OUT], AF.Relu,
                                     scale=rec[:, 0:1])
            else:
                nc.vector.tensor_scalar(rt[:], op[:, 0:F_OUT], rec[:, 0:1],
                                        0.0, op0=ALU.mult, op1=ALU.max)
            if h == 0:
                nc.vector.scalar_tensor_tensor(s_acc[it][:], zt[:], 1.0,
                                               rt[:], op0=ALU.min,
                                               op1=ALU.add)
            else:
                ut = ep.tile([P, F_OUT], BF16, name="ut", tag="ut")
                nc.vector.scalar_tensor_tensor(ut[:], zt[:], 1.0, rt[:],
                                               op0=ALU.min, op1=ALU.add)
                nc.vector.tensor_add(s_acc[it][:], s_acc[it][:], ut[:])

    for h in range(H):
        # broadcast the needed f1-derived rows along partitions (GPSIMD)
        z1sep = h < z1sep_n
        z2sep = h < z2sep_n
        f1b = ef1b = ef1b02 = None
        if not (z1sep and z2sep):
            f1b = f1bp.tile([P, N], BF16, name="f1b", tag="f1b")
            nc.gpsimd.partition_broadcast(
                f1b[:], f1flat[0:1, h * N:(h + 1) * N])
        if z1sep:
            ef1b = f1bp.tile([P, N], BF16, name="ef1b", tag="ef1b")
            nc.gpsimd.partition_broadcast(
                ef1b[:], ef1flat[0:1, h * N:(h + 1) * N])
        if z2sep:
            ef1b02 = f1bp.tile([P, N], BF16, name="ef1b02", tag="ef1b02")
            nc.gpsimd.partition_broadcast(
                ef1b02[:], ef1flat02[0:1, h * N:(h + 1) * N])
        atts = []
        for q in range(NT // 2):
            z1 = zp.tile([P, 2, N], BF16, name="z1", tag="z1")
            z2 = zp.tile([P, 2, N], BF16, name="z2", tag="z2")
            att = att_p.tile([P, 2, N], BF16, name="att", tag="att")
            for s in range(2):
                jt = 2 * q + s
                if z1sep:
                    nc.vector.tensor_scalar(z1[:, s, :], ef1b[:],
                                            ef2[jt][:, h:h + 1], None,
                                            op0=ALU.mult)
                else:
                    nc.scalar.activation(z1[:, s, :], f1b[:], AF.Exp,
                                         bias=f12[jt][:, 8 + h:9 + h],
                                         scale=1.0)
                if z2sep:
                    nc.vector.tensor_scalar(z2[:, s, :], ef1b02[:],
                                            ef2[jt][:, 8 + h:9 + h], None,
                                            op0=ALU.mult)
                elif z_mode == "z2pool":
                    nc.gpsimd.tensor_scalar(z2[:, s, :].bitcast(I16), f1b[:],
                                            A02, bb[jt][:, h:h + 1],
                                            op0=ALU.mult, op1=ALU.add)
                else:
                    nc.scalar.activation(z2[:, s, :], f1b[:], AF.Exp,
                                         bias=bb[jt][:, h:h + 1], scale=0.2)
            tm = tmp_p.tile([P, 2, N], BF16, name="tm", tag="tm")
            nc.vector.tensor_max(tm[:], z1[:], z2[:])
            nc.vector.tensor_mul(att[:], tm[:], adjT[q][:])
            atts.append(att)
        output_stage(h, atts)

    if "Cz" in variant or not on("D"):
        dummy_out()
        return

    # ---------------- Stage D: log_softmax over F_OUT (batched phases so
    # the ACT table switches Exp->Ln only once) ----------------
    dps = ctx.enter_context(tc.tile_pool(name="lsm_s", bufs=8))
    dpz = ctx.enter_context(tc.tile_pool(name="lsm_z", bufs=2))
    dpo = ctx.enter_context(tc.tile_pool(name="lsm_o", bufs=4))
    dss, lnds = [], []
    for it in range(NT):
        zz = dpz.tile([P, F_OUT], F32, name="zz", tag="zz")
        ds = dps.tile([P, 1], F32, name="ds", tag="ds")
        nc.scalar.activation(zz[:], s_acc[it][:], AF.Exp,
                             accum_out=ds[:, 0:1])
        dss.append(ds)
    for it in range(NT):
        lnd = dps.tile([P, 1], F32, name="lnd", tag="lnd")
        nc.scalar.activation(lnd[:], dss[it][:], AF.Ln)
        lnds.append(lnd)
    for it in range(NT):
        ot_t = dpo.tile([P, F_OUT], F32, name="outt", tag="outt")
        nc.vector.tensor_scalar(ot_t[:], s_acc[it][:], lnds[it][:, 0:1],
                                None, op0=ALU.subtract)
        nc.sync.dma_start(out_d[it * P:(it + 1) * P, :], ot_t[:])


_PROGRAM_CACHE = {}


def build_gat_program(repeats=1, variant=()):
    key = ("nc", repeats, tuple(sorted(variant)))
    if key in _PROGRAM_CACHE:
        return _PROGRAM_CACHE[key]
    nc = bacc.Bacc("TRN2", debug=False)
    x_d = nc.dram_tensor("x", (N, F_IN), F32, kind="ExternalInput").ap()
    adj_d = nc.dram_tensor("adj", (N, N), I32, kind="ExternalInput").ap()
    W_d = nc.dram_tensor("W", (H, F_OUT, F_IN), F32, kind="ExternalInput").ap()
    a1_d = nc.dram_tensor("a1", (H, F_OUT), F32, kind="ExternalInput").ap()
    a2_d = nc.dram_tensor("a2", (H, F_OUT), F32, kind="ExternalInput").ap()
    out_d = nc.dram_tensor("out", (N, F_OUT), F32, kind="ExternalOutput").ap()
    with tile.TileContext(nc) as tc:
        for _ in range(repeats):
            gat_kernel(tc, out_d, x_d, adj_d, W_d, a1_d, a2_d, variant=variant)
    nc.compile()
    _PROGRAM_CACHE[key] = nc
    return nc


def kernel(x, adj, W, a1, a2, _trace=False):
    from concourse.bass_utils import run_bass_kernel_spmd

    x = np.ascontiguousarray(np.asarray(x, dtype=np.float32))
    adj = np.ascontiguousarray(np.asarray(adj, dtype=np.int32))
    W = np.ascontiguousarray(np.asarray(W, dtype=np.float32))
    a1 = np.ascontiguousarray(np.asarray(a1, dtype=np.float32))
    a2 = np.ascontiguousarray(np.asarray(a2, dtype=np.float32))

    nc = build_gat_program()
    in_maps = [{"x": x[b], "adj": adj[b], "W": W, "a1": a1, "a2": a2}
               for b in range(B)]
    res = run_bass_kernel_spmd(nc, in_maps, core_ids=list(range(B)),
                               trace=_trace)
    out = np.stack([res.results[b]["out"] for b in range(B)])
    if _trace:
        kernel.last_result = res
    return out
